# revision 1
# baseline (speedup 1.0000x reference)
"""Multi-head attention (B=2, N=2048, D=1024, H=16, hd=64) on 8 TRN2 NeuronCores.

Sharding: data-parallel over batch (2) x tensor-parallel over heads (4 groups
of 4 heads). Each core computes, for its (batch b, head group g), the partial
output  outT_c[e, i] = sum_{d in shard} Wo[e, d] * O[i, d]  over its 256
sharded head dims; the host sums the 4 head-group partials per batch, adds bo.

Per-core device kernel (all matmuls float32r, ~2e-4 rel err):
  xT (c,i) resident in SBUF.  Q^T = WqT.T @ xT  (natural, 2 heads per 128-row
  tile),  K^T = (Wk*scale)T.T @ xT stored in per-head 128-row slots whose
  other 64 rows are zero (so S^T can use full K=128 matmuls),  V = xT.T @ WvT
  with an appended ones column so PV also accumulates Z = sum_j exp(S) in
  PSUM row 64.  Per (i-half, head):  S^T[jb,:] = ktp[:,h,jb].T @ qt[:,h//2,:],
  ACT exp PSUM->SBUF (f32r),  PV accumulates O^T[65, 1024] over the 16 jb.
  O^T is scaled by 1/Z (fast reciprocal + DMA broadcast through DRAM), then
  the Wo projection writes outT[1024, 2048] to DRAM.
"""
import sys

sys.path.insert(0, "/opt/trn_rl_repo")

import numpy as np

import concourse.bass as bass
import concourse.tile as tile
from concourse import bacc, bass_utils, mybir

P = 128
NTOK = 2048          # sequence length
D = 1024             # model dim
HPC = 4              # heads per core
HD = 64              # head dim
DSH = HPC * HD       # 256: sharded head dims per core
CO = D // P          # 8 contraction chunks over c
NIH = 2              # i halves
IHW = NTOK // NIH    # 1024
NC2 = IHW // 512     # 512-chunks per half = 2
NJB = NTOK // P      # 16 j blocks
SCALE = HD ** -0.5

F32 = mybir.dt.float32
F32R = mybir.dt.float32r
DT_NP = np.float32


def build_nc():
    nc = bacc.Bacc("TRN2", target_bir_lowering=False, debug=False)

    xt_d = nc.dram_tensor("xt", [D, NTOK], F32R, kind="ExternalInput").ap()
    wqt_d = nc.dram_tensor("wqt", [D, DSH], F32R, kind="ExternalInput").ap()
    wkt_d = nc.dram_tensor("wkt", [D, DSH], F32R, kind="ExternalInput").ap()
    wvt_d = nc.dram_tensor("wvt", [D, DSH], F32R, kind="ExternalInput").ap()
    wot_d = nc.dram_tensor("wot", [DSH, D], F32R, kind="ExternalInput").ap()
    zpad_d = nc.dram_tensor("zpad", [64, NTOK], F32R, kind="ExternalInput").ap()
    vones_d = nc.dram_tensor("vones", [P, NJB, HPC, 1], F32R,
                             kind="ExternalInput").ap()
    outt_d = nc.dram_tensor("outt", [D, NTOK], F32, kind="ExternalOutput").ap()

    xt_t = xt_d.rearrange("(o p) i -> p o i", p=P)        # [128, 8, 2048]
    wq_t = wqt_d.rearrange("(o p) d -> p o d", p=P)       # [128, 8, 256]
    wk_t = wkt_d.rearrange("(o p) d -> p o d", p=P)
    wv_t = wvt_d.rearrange("(o p) d -> p o d", p=P)
    wo_t = wot_d.rearrange("(o p) e -> p o e", p=P)       # [128, 2, 1024]
    out_t = outt_d.rearrange("(m p) i -> p m i", p=P)     # [128, 8, 2048]

    with tile.TileContext(nc) as tc:
        with (
            tc.tile_pool(name="sbp", bufs=1) as sbp,           # persistent
            tc.tile_pool(name="ps", bufs=1, space="PSUM") as ps,
            tc.tile_pool(name="dr", bufs=2, space="DRAM") as dr,
        ):
            # persistent activation tensors
            qt = sbp.tile([P, 2, NTOK], F32R, tag="qt")        # Q^T natural
            ktp = sbp.tile([P, HPC, NTOK], F32R, tag="ktp")    # K^T padded
            vaug = sbp.tile([P, NJB, HPC, 65], F32R, tag="vaug")  # V | ones
            ota = sbp.tile([P, 2, NTOK], F32R, tag="ota")      # O^T all heads
            wo = sbp.tile([P, 2, D], F32R, tag="wo")
            nc.sync.dma_start(wo[:], wo_t)

            # zero rows of the padded K^T slots; ones column of vaug
            nc.sync.dma_start(ktp[64:128, 0, :], zpad_d)
            nc.sync.dma_start(ktp[0:64, 1, :], zpad_d)
            nc.sync.dma_start(ktp[64:128, 2, :], zpad_d)
            nc.sync.dma_start(ktp[0:64, 3, :], zpad_d)
            nc.sync.dma_start(vaug[:, :, :, 64:65], vones_d)

            # psum tags: a0/a1 [128,1024] (2 banks each), o0..o3 (1 bank each)
            def psA(i):
                return ps.tile([P, IHW], F32, tag=f"a{i % 2}", name=f"psA{i % 2}")

            def psO(i, shape=(65, 512)):
                return ps.tile(list(shape), F32, tag=f"o{i % 4}", name=f"psO{i % 4}")

            # ------------- phase 1: loads + projections (xt/w scoped) -------
            with tc.tile_pool(name="sbl", bufs=1) as sbl:
                xt = []
                for o in range(CO):
                    t = sbl.tile([P, NTOK], F32R, tag=f"xt{o}")
                    nc.sync.dma_start(t[:], xt_t[:, o, :])
                    xt.append(t)
                wq = sbl.tile([P, CO, DSH], F32R, tag="wq")
                wk = sbl.tile([P, CO, DSH], F32R, tag="wk")
                wv = sbl.tile([P, CO, DSH], F32R, tag="wv")
                nc.sync.dma_start(wq[:], wq_t)
                nc.sync.dma_start(wk[:], wk_t)
                nc.sync.dma_start(wv[:], wv_t)

                # Q^T / K^T projections (2 heads per 128-row M tile)
                ai = 0
                for w_sb, is_q in ((wq, True), (wk, False)):
                    for mt in range(2):
                        for ih in range(NIH):
                            pp = psA(ai)
                            ai += 1
                            for c in range(NC2):
                                for o in range(CO):
                                    nc.tensor.matmul(
                                        pp[:, c * 512:(c + 1) * 512],
                                        w_sb[:, o, mt * P:(mt + 1) * P],
                                        xt[o][:, ih * IHW + c * 512:
                                              ih * IHW + (c + 1) * 512],
                                        start=(o == 0), stop=(o == CO - 1),
                                    )
                            sl = slice(ih * IHW, (ih + 1) * IHW)
                            if is_q:
                                nc.scalar.copy(qt[:, mt, sl], pp[:, :])
                            else:
                                nc.scalar.copy(ktp[0:64, 2 * mt, sl], pp[0:64, :])
                                nc.scalar.copy(ktp[64:128, 2 * mt + 1, sl],
                                               pp[64:128, :])

                # V projection
                for it in range(NJB):
                    pv = psO(it, (P, 512))
                    for o in range(CO):
                        nc.tensor.matmul(
                            pv[:, 0:DSH],
                            xt[o][:, it * P:(it + 1) * P],
                            wv[:, o, :],
                            start=(o == 0), stop=(o == CO - 1),
                        )
                    nc.scalar.copy(
                        vaug[:, it, :, 0:64],
                        pv[:, 0:DSH].rearrange("p (h d) -> p h d", d=HD),
                    )

            # ------------- phase 2: attention + output projection -----------
            with tc.tile_pool(name="sbw", bufs=1) as sbw:
                for ih in range(NIH):
                    isl = slice(ih * IHW, (ih + 1) * IHW)
                    for pair in range(2):
                        heads = (2 * pair, 2 * pair + 1)
                        pso = {}
                        for k, h in enumerate(heads):
                            for c in range(NC2):
                                pso[(h, c)] = psO(2 * k + c)
                        for jb in range(NJB):
                            pst = {}
                            for k, h in enumerate(heads):
                                pa = psA(k)
                                pst[h] = pa
                                for c in range(NC2):
                                    nc.tensor.matmul(
                                        pa[:, c * 512:(c + 1) * 512],
                                        ktp[:, h, jb * P:(jb + 1) * P],
                                        qt[:, h // 2, ih * IHW + c * 512:
                                           ih * IHW + (c + 1) * 512],
                                        start=True, stop=True,
                                    )
                            for k, h in enumerate(heads):
                                es = sbw.tile([P, IHW], F32R, tag="es", bufs=6)
                                nc.scalar.activation(
                                    es[:], pst[h][:],
                                    mybir.ActivationFunctionType.Exp,
                                )
                                for c in range(NC2):
                                    nc.tensor.matmul(
                                        pso[(h, c)][:],
                                        vaug[:, jb, h, 0:65],
                                        es[:, c * 512:(c + 1) * 512],
                                        start=(jb == 0), stop=(jb == NJB - 1),
                                    )
                        # normalize: O^T[h] = pso rows 0:64 times 1/Z.
                        # Evacuate PSUM first (frees the PV banks for the next
                        # pair ~4us earlier); recip/broadcast/mult then run off
                        # the critical path against the SBUF copy.
                        for k, h in enumerate(heads):
                            ot = sbw.tile([64, IHW], F32, tag="otmp", bufs=2)
                            zt = sbw.tile([1, IHW], F32, tag="zt", bufs=2)
                            for c in range(NC2):
                                nc.vector.tensor_copy(
                                    ot[:, c * 512:(c + 1) * 512],
                                    pso[(h, c)][0:64, :])
                                nc.vector.tensor_copy(
                                    zt[:, c * 512:(c + 1) * 512],
                                    pso[(h, c)][64:65, :])
                            rt = sbw.tile([1, IHW], F32, tag="rt", bufs=2)
                            nc.vector.reciprocal_approx_fast(out=rt[:], in_=zt[:])
                            rdram = dr.tile([1, IHW], F32, tag="rd")
                            nc.sync.dma_start(rdram[:], rt[:])
                            rb = sbw.tile([64, IHW], F32, tag="rb", bufs=2)
                            nc.sync.dma_start(rb[:], rdram[:].to_broadcast((64, IHW)))
                            row = slice((h % 2) * 64, (h % 2) * 64 + 64)
                            for c in range(NC2):
                                nc.vector.tensor_mul(
                                    ota[row, h // 2, ih * IHW + c * 512:
                                        ih * IHW + (c + 1) * 512],
                                    ot[:, c * 512:(c + 1) * 512],
                                    rb[:, c * 512:(c + 1) * 512],
                                )
                    # output projection for this ih half.  Uses the o-tag
                    # PSUM banks (free between pairs) so the a-tags stay
                    # available for the next window's S^T matmuls.
                    for mt in range(8):
                        stg = sbw.tile([P, IHW], F32, tag="stg", bufs=2)
                        for c in range(NC2):
                            pe = psO(mt * NC2 + c, (P, 512))
                            for o in range(2):
                                nc.tensor.matmul(
                                    pe[:],
                                    wo[:, o, mt * P:(mt + 1) * P],
                                    ota[:, o, ih * IHW + c * 512:
                                        ih * IHW + (c + 1) * 512],
                                    start=(o == 0), stop=(o == 1),
                                )
                            nc.vector.tensor_copy(
                                stg[:, c * 512:(c + 1) * 512], pe[:])
                        nc.sync.dma_start(out_t[:, mt, isl], stg[:])

    nc.compile()
    return nc


_NC_CACHE = None


def _get_nc():
    global _NC_CACHE
    if _NC_CACHE is None:
        _NC_CACHE = build_nc()
    return _NC_CACHE


def kernel(x, Wq, Wk, Wv, Wo, bo, _trace=False):
    x = np.asarray(x, dtype=DT_NP)
    Wq = np.asarray(Wq, dtype=DT_NP)
    Wk = np.asarray(Wk, dtype=DT_NP)
    Wv = np.asarray(Wv, dtype=DT_NP)
    Wo = np.asarray(Wo, dtype=DT_NP)
    bo = np.asarray(bo, dtype=DT_NP)
    B = x.shape[0]

    nc = _get_nc()
    zpad = np.zeros((64, NTOK), dtype=DT_NP)
    vones = np.ones((P, NJB, HPC, 1), dtype=DT_NP)
    in_maps = []
    for core in range(8):
        b, hg = divmod(core, 4)
        rows = slice(hg * DSH, (hg + 1) * DSH)
        in_maps.append({
            "xt": np.ascontiguousarray(x[b].T),
            "wqt": np.ascontiguousarray(Wq[rows, :].T),
            "wkt": np.ascontiguousarray((Wk[rows, :] * SCALE).T),
            "wvt": np.ascontiguousarray(Wv[rows, :].T),
            "wot": np.ascontiguousarray(Wo[:, rows].T),
            "zpad": zpad,
            "vones": vones,
        })

    res = bass_utils.run_bass_kernel_spmd(
        nc, in_maps, core_ids=list(range(8)), trace=_trace)

    out = np.zeros((B, NTOK, D), dtype=DT_NP)
    for core in range(8):
        b = core // 4
        out[b] += res.results[core]["outt"].T
    out += bo
    if _trace:
        kernel.last_results = res
    return out



# revision 4
# speedup vs baseline: 1.3396x; 1.3396x over previous
"""Multi-head attention (B=2, N=2048, D=1024, H=16, hd=64) on 8 TRN2 NeuronCores.

Sharding: data-parallel over batch (2) x tensor-parallel over heads (4 groups
of 4 heads). Each core computes, for its (batch b, head group g), the partial
output  outT_c[e, i] = sum_{d in shard} Wo[e, d] * O[i, d]  over its 256
sharded head dims; the host sums the 4 head-group partials per batch, adds bo.

v2: all matmul operands bf16 (halves DMA + SBUF traffic), single-head
attention windows (h, ih) with the QKV/O projections hand-interleaved into
the PE slack of the ACT(exp)-bound attention pipeline.  Per jb slot:
S^T (2 matmuls) -> EXP (ACT, psum->sbuf bf16) -> PV accumulate, with V/K/Q/
out-proj "filler" fills placed so their results land just before first use.
PSUM: s0/s1 [128,1024] score double-buffer, v0/v1 [65,512] PV+Z accum,
p0/p1 [128,512] projection scratch.  exp sum Z rides in PV row 64 via a ones
column appended to V (memset on device, not DMA'd).
"""
import sys

sys.path.insert(0, "/opt/trn_rl_repo")

import ml_dtypes
import numpy as np

import concourse.bass as bass
import concourse.tile as tile
from concourse import bacc, bass_utils, mybir

P = 128
NTOK = 2048          # sequence length
D = 1024             # model dim
HPC = 4              # heads per core
HD = 64              # head dim
DSH = HPC * HD       # 256: sharded head dims per core
CO = 8               # contraction chunks over c (D/P)
NIH = 2              # i halves
IHW = NTOK // NIH    # 1024
NJB = NTOK // P      # 16 j blocks
SCALE = HD ** -0.5

F32 = mybir.dt.float32
BF16 = mybir.dt.bfloat16
NP_BF16 = ml_dtypes.bfloat16
EXP_FN = mybir.ActivationFunctionType.Exp


def build_nc():
    nc = bacc.Bacc("TRN2", target_bir_lowering=False, debug=False)

    xt_d = nc.dram_tensor("xt", [D, NTOK], BF16, kind="ExternalInput").ap()
    wqt_d = nc.dram_tensor("wqt", [D, DSH], BF16, kind="ExternalInput").ap()
    wkt_d = nc.dram_tensor("wkt", [D, DSH], BF16, kind="ExternalInput").ap()
    wvt_d = nc.dram_tensor("wvt", [D, DSH], BF16, kind="ExternalInput").ap()
    wot_d = nc.dram_tensor("wot", [DSH, D], BF16, kind="ExternalInput").ap()
    zpad_d = nc.dram_tensor("zpad", [64, NTOK], BF16, kind="ExternalInput").ap()
    outt_d = nc.dram_tensor("outt", [D, NTOK], BF16, kind="ExternalOutput").ap()

    xt_t = xt_d.rearrange("(o p) i -> p o i", p=P)        # [128, 8, 2048]
    wq_t = wqt_d.rearrange("(o p) d -> p o d", p=P)       # [128, 8, 256]
    wk_t = wkt_d.rearrange("(o p) d -> p o d", p=P)
    wv_t = wvt_d.rearrange("(o p) d -> p o d", p=P)
    wo_t = wot_d.rearrange("(o p) e -> p o e", p=P)       # [128, 2, 1024]
    out_t = outt_d.rearrange("(m p) i -> p m i", p=P)     # [128, 8, 2048]

    with tile.TileContext(nc) as tc:
        with (
            tc.tile_pool(name="sbp", bufs=1) as sbp,           # persistent
            tc.tile_pool(name="sbw", bufs=1) as sbw,           # working
            tc.tile_pool(name="ps", bufs=1, space="PSUM") as ps,
            tc.tile_pool(name="dr", bufs=2, space="DRAM") as dr,
        ):
            # ---------------- persistent tiles ----------------
            qt = sbp.tile([P, 2, NTOK], BF16, tag="qt")        # Q^T natural
            ktp = sbp.tile([P, HPC, NTOK], BF16, tag="ktp")    # K^T padded
            vaug = sbp.tile([P, NJB, HPC, 65], BF16, tag="vaug")  # V | ones
            ota = sbp.tile([P, 2, NTOK], BF16, tag="ota")      # O^T all heads
            wo = sbp.tile([P, 2, D], BF16, tag="wo")
            wq = sbp.tile([P, CO, DSH], BF16, tag="wq")
            wk = sbp.tile([P, CO, DSH], BF16, tag="wk")
            wv = sbp.tile([P, CO, DSH], BF16, tag="wv")
            # x column halves: xh[o][ih] = x^T rows o*128.. , cols ih*1024..
            xh = [[sbp.tile([P, IHW], BF16, tag=f"xh{o}_{i}", name=f"xh{o}_{i}")
                   for i in range(2)] for o in range(CO)]
            # pair-0 partial of the ih1 output projection (f32 staging)
            stg0 = [sbp.tile([P, IHW], F32, tag=f"stg0_{mt}", name=f"stg0_{mt}")
                    for mt in range(8)]

            # ---------------- DMA issue (arrival order matters) -------------
            nc.sync.dma_start(ktp[64:128, 0, :], zpad_d)
            nc.sync.dma_start(ktp[0:64, 1, :], zpad_d)
            nc.sync.dma_start(ktp[64:128, 2, :], zpad_d)
            nc.sync.dma_start(ktp[0:64, 3, :], zpad_d)
            nc.sync.dma_start(wk[:], wk_t)
            nc.sync.dma_start(wq[:], wq_t)
            nc.sync.dma_start(wv[:], wv_t)
            for o in range(CO):
                nc.sync.dma_start(xh[o][0][:], xt_t[:, o, 0:IHW])
            for o in range(CO):
                nc.sync.dma_start(xh[o][1][:], xt_t[:, o, IHW:NTOK])
            nc.sync.dma_start(wo[:], wo_t)
            # ones column of V-augmented (device-side, avoids 8K descriptors)
            nc.vector.memset(vaug[:, :, :, 64:65], 1.0)

            # ---------------- filler builders ----------------
            pcycle = [0]

            def ptag():
                pcycle[0] += 1
                return f"p{pcycle[0] % 2}"

            def kq_fill(is_q, mt, ihh, c):
                """One [128,512] projection fill: Q^T/K^T for heads 2mt,2mt+1
                over token slice (ihh, c)."""
                t = ptag()
                pp = ps.tile([P, 512], F32, tag=t, name=f"ps_{t}")
                w_sb = wq if is_q else wk
                for o in range(CO):
                    nc.tensor.matmul(
                        pp[:],
                        w_sb[:, o, mt * P:(mt + 1) * P],
                        xh[o][ihh][:, c * 512:(c + 1) * 512],
                        start=(o == 0), stop=(o == CO - 1),
                    )
                sl = slice(ihh * IHW + c * 512, ihh * IHW + (c + 1) * 512)
                if is_q:
                    nc.vector.tensor_copy(qt[:, mt, sl], pp[:])
                else:
                    nc.vector.tensor_copy(ktp[0:64, 2 * mt, sl], pp[0:64, :])
                    nc.vector.tensor_copy(ktp[64:128, 2 * mt + 1, sl],
                                          pp[64:128, :])

            def v_fill(it, pair):
                """V projection for token block it, head pair `pair`."""
                t = ptag()
                pp = ps.tile([P, P], F32, tag=t, name=f"ps_{t}")
                ihh, loc = divmod(it, 8)
                for o in range(CO):
                    nc.tensor.matmul(
                        pp[:],
                        xh[o][ihh][:, loc * P:(loc + 1) * P],
                        wv[:, o, pair * P:(pair + 1) * P],
                        start=(o == 0), stop=(o == CO - 1),
                    )
                nc.vector.tensor_copy(
                    vaug[:, it, 2 * pair:2 * pair + 2, 0:64],
                    pp[:].rearrange("p (h d) -> p h d", d=HD),
                )

            def oproj_fill(mt, ihh, c, stg, mode="full"):
                """Output projection fill [128,512] for row-tile mt, token
                slice (ihh, c).  mode: full = both pairs; part0 = pair-0 only
                (staged f32); fin1 = pair-1 only, added to the staged half."""
                t = ptag()
                pp = ps.tile([P, 512], F32, tag=t, name=f"ps_{t}")
                os_ = (0, 1) if mode == "full" else ((0,) if mode == "part0" else (1,))
                for o in os_:
                    nc.tensor.matmul(
                        pp[:],
                        wo[:, o, mt * P:(mt + 1) * P],
                        ota[:, o, ihh * IHW + c * 512: ihh * IHW + (c + 1) * 512],
                        start=(o == os_[0]), stop=(o == os_[-1]),
                    )
                cs = slice(c * 512, (c + 1) * 512)
                if mode == "part0":
                    nc.vector.tensor_copy(stg0[mt][:, cs], pp[:])
                elif mode == "fin1":
                    nc.vector.tensor_add(stg[:, cs], stg0[mt][:, cs], pp[:])
                else:
                    nc.vector.tensor_copy(stg[:, cs], pp[:])

            def oproj_mt(mt, ihh, mode="full"):
                """Output-projection row-tile (+ store unless staging)."""
                stg = None
                if mode != "part0":
                    stg = sbw.tile([P, IHW], BF16, tag="stg", bufs=2, name="stg")
                for c in range(2):
                    oproj_fill(mt, ihh, c, stg, mode)
                if mode != "part0":
                    nc.sync.dma_start(
                        out_t[:, mt, ihh * IHW:(ihh + 1) * IHW], stg[:])

            # ---------------- attention window ----------------
            def attn_window(h, ihh, pre, mid):
                """One head x one i-half.  pre[jb]/mid[jb]: filler callables
                issued at the start of slot jb / between S and PV."""
                vc = [ps.tile([65, 512], F32, tag=f"v{c}", name=f"ps_v{c}")
                      for c in range(2)]
                for jb in range(NJB):
                    for f in pre.get(jb, ()):
                        f()
                    ss = ps.tile([P, IHW], F32, tag=f"s{jb % 2}",
                                 name=f"ps_s{jb % 2}")
                    for c in range(2):
                        nc.tensor.matmul(
                            ss[:, c * 512:(c + 1) * 512],
                            ktp[:, h, jb * P:(jb + 1) * P],
                            qt[:, h // 2, ihh * IHW + c * 512:
                               ihh * IHW + (c + 1) * 512],
                            start=True, stop=True,
                        )
                    es = sbw.tile([P, IHW], BF16, tag="es", bufs=4, name="es")
                    nc.scalar.activation(es[:], ss[:], EXP_FN)
                    for f in mid.get(jb, ()):
                        f()
                    for c in range(2):
                        nc.tensor.matmul(
                            vc[c][:],
                            vaug[:, jb, h, 0:65],
                            es[:, c * 512:(c + 1) * 512],
                            start=(jb == 0), stop=(jb == NJB - 1),
                        )
                # normalize: O^T[h] = PV rows 0:64 times 1/Z (PV row 64).
                ot = sbw.tile([64, IHW], F32, tag="ot", bufs=2, name="ot")
                zt = sbw.tile([1, IHW], F32, tag="zt", bufs=2, name="zt")
                for c in range(2):
                    cs = slice(c * 512, (c + 1) * 512)
                    nc.vector.tensor_copy(ot[:, cs], vc[c][0:64, :])
                    nc.vector.tensor_copy(zt[:, cs], vc[c][64:65, :])
                rt = sbw.tile([1, IHW], F32, tag="rt", bufs=2, name="rt")
                nc.vector.reciprocal_approx_fast(out=rt[:], in_=zt[:])
                rdram = dr.tile([1, IHW], F32, tag="rd")
                nc.sync.dma_start(rdram[:], rt[:])
                rb = sbw.tile([64, IHW], F32, tag="rb", bufs=2, name="rb")
                nc.sync.dma_start(rb[:], rdram[:].to_broadcast((64, IHW)))
                row = slice((h % 2) * 64, (h % 2) * 64 + 64)
                isl = slice(ihh * IHW, (ihh + 1) * IHW)
                nc.vector.tensor_mul(ota[row, h // 2, isl], ot[:], rb[:])

            # ---------------- schedule ----------------
            # lead-in: K^T for heads 0,1 over j 0:512; Q^T heads 0,1 ih0
            kq_fill(False, 0, 0, 0)
            kq_fill(True, 0, 0, 0)
            kq_fill(True, 0, 0, 1)

            FB = lambda *fs: tuple(fs)

            # w0 (ih0, h0): remaining K0 quarters paced; V pair0 per jb
            pre0 = {1: FB(lambda: kq_fill(False, 0, 0, 1)),
                    5: FB(lambda: kq_fill(False, 0, 1, 0)),
                    9: FB(lambda: kq_fill(False, 0, 1, 1))}
            mid0 = {jb: FB(lambda jb=jb: v_fill(jb, 0)) for jb in range(NJB)}
            attn_window(0, 0, pre0, mid0)

            # w1 (ih0, h1): V pair1 per jb; Q1 ih0; first K1 quarter
            pre1 = {0: FB(lambda: kq_fill(True, 1, 0, 0)),
                    2: FB(lambda: kq_fill(True, 1, 0, 1)),
                    12: FB(lambda: kq_fill(False, 1, 0, 0))}
            mid1 = {jb: FB(lambda jb=jb: v_fill(jb, 1)) for jb in range(NJB)}
            attn_window(1, 0, pre1, mid1)

            # w2 (ih0, h2): K1 quarters paced one quarter ahead of use
            pre2 = {0: FB(lambda: kq_fill(False, 1, 0, 1)),
                    4: FB(lambda: kq_fill(False, 1, 1, 0)),
                    8: FB(lambda: kq_fill(False, 1, 1, 1))}
            attn_window(2, 0, pre2, {})

            # w3 (ih0, h3): Q0 ih1 (due w4)
            pre3 = {0: FB(lambda: kq_fill(True, 0, 1, 0)),
                    4: FB(lambda: kq_fill(True, 0, 1, 1))}
            attn_window(3, 0, pre3, {})

            # w4..w7 (ih1, h0..h3): Q1 ih1 (due w6), out-proj(ih0) spread,
            # pair-0 partials of out-proj(ih1) in w6/w7.
            pre4 = {0: FB(lambda: kq_fill(True, 1, 1, 0)),
                    4: FB(lambda: kq_fill(True, 1, 1, 1)),
                    8: FB(lambda: oproj_mt(0, 0)),
                    12: FB(lambda: oproj_mt(1, 0))}
            attn_window(0, 1, pre4, {})

            pre5 = {jb: FB(lambda mt=2 + jb // 4: oproj_mt(mt, 0))
                    for jb in (0, 4, 8, 12)}
            attn_window(1, 1, pre5, {})

            pre6 = {0: FB(lambda: oproj_mt(6, 0)),
                    4: FB(lambda: oproj_mt(7, 0)),
                    8: FB(lambda: oproj_mt(0, 1, "part0"),
                          lambda: oproj_mt(1, 1, "part0")),
                    12: FB(lambda: oproj_mt(2, 1, "part0"),
                           lambda: oproj_mt(3, 1, "part0"))}
            attn_window(2, 1, pre6, {})

            pre7 = {0: FB(lambda: oproj_mt(4, 1, "part0")),
                    4: FB(lambda: oproj_mt(5, 1, "part0")),
                    8: FB(lambda: oproj_mt(6, 1, "part0")),
                    12: FB(lambda: oproj_mt(7, 1, "part0"))}
            attn_window(3, 1, pre7, {})

            # tail: finish ih1 out-projection with the pair-1 contribution
            for mt in range(8):
                oproj_mt(mt, 1, "fin1")

    nc.compile()
    return nc


_NC_CACHE = None


def _get_nc():
    global _NC_CACHE
    if _NC_CACHE is None:
        _NC_CACHE = build_nc()
    return _NC_CACHE


def kernel(x, Wq, Wk, Wv, Wo, bo, _trace=False):
    x = np.asarray(x, dtype=np.float32)
    Wq = np.asarray(Wq, dtype=np.float32)
    Wk = np.asarray(Wk, dtype=np.float32)
    Wv = np.asarray(Wv, dtype=np.float32)
    Wo = np.asarray(Wo, dtype=np.float32)
    bo = np.asarray(bo, dtype=np.float32)
    B = x.shape[0]

    nc = _get_nc()
    zpad = np.zeros((64, NTOK), dtype=NP_BF16)
    in_maps = []
    for core in range(8):
        b, hg = divmod(core, 4)
        rows = slice(hg * DSH, (hg + 1) * DSH)
        in_maps.append({
            "xt": np.ascontiguousarray(x[b].T).astype(NP_BF16),
            "wqt": np.ascontiguousarray(Wq[rows, :].T).astype(NP_BF16),
            "wkt": np.ascontiguousarray((Wk[rows, :] * SCALE).T).astype(NP_BF16),
            "wvt": np.ascontiguousarray(Wv[rows, :].T).astype(NP_BF16),
            "wot": np.ascontiguousarray(Wo[:, rows].T).astype(NP_BF16),
            "zpad": zpad,
        })

    res = bass_utils.run_bass_kernel_spmd(
        nc, in_maps, core_ids=list(range(8)), trace=_trace)

    out = np.zeros((B, NTOK, D), dtype=np.float32)
    for core in range(8):
        b = core // 4
        out[b] += res.results[core]["outt"].astype(np.float32).T
    out += bo
    if _trace:
        kernel.last_results = res
    return out


# revision 9
# speedup vs baseline: 1.3603x; 1.0154x over previous
"""Multi-head attention (B=2, N=2048, D=1024, H=16, hd=64) on 8 TRN2 NeuronCores.

Sharding: data-parallel over batch (2) x tensor-parallel over heads (4 groups
of 4 heads). Each core computes, for its (batch b, head group g), the partial
output  outT_c[e, i] = sum_{d in shard} Wo[e, d] * O[i, d]  over its 256
sharded head dims; the host sums the 4 head-group partials per batch, adds bo.

v3: bf16 operands, host-preswizzled DRAM layouts (one contiguous descriptor
per partition per DMA), single-head attention windows (h, ih) with QKV/O
projection fills hand-interleaved into the PE slack of the ACT(exp)-bound
attention pipeline.  Per jb slot: S^T (2 x K=64 matmuls) -> EXP -> PV of the
PREVIOUS jb (software pipelining hides the exp->PV latency).  PSUM: s0/s1
[128,1024] score double-buffer, v0/v1 [65,512] PV+Z accumulators, p0/p1
[128,512] projection scratch.  Z rides in PV row 64 via a ones column
(memset).  Tail: last-window 1/Z broadcast via a K=1 PE matmul (no DRAM
roundtrip); final out-projection accumulates in the freed s-banks with
evacuation alternating between the idle ACT and DVE engines.
"""
import sys

sys.path.insert(0, "/opt/trn_rl_repo")

import ml_dtypes
import numpy as np

import concourse.bass as bass
import concourse.tile as tile
from concourse import bacc, bass_utils, mybir

P = 128
NTOK = 2048          # sequence length
D = 1024             # model dim
HPC = 4              # heads per core
HD = 64              # head dim
DSH = HPC * HD       # 256: sharded head dims per core
CO = 8               # contraction chunks over c (D/P)
NIH = 2              # i halves
IHW = NTOK // NIH    # 1024
NJB = NTOK // P      # 16 j blocks
SCALE = HD ** -0.5

F32 = mybir.dt.float32
BF16 = mybir.dt.bfloat16
NP_BF16 = ml_dtypes.bfloat16
EXP_FN = mybir.ActivationFunctionType.Exp


def build_nc():
    nc = bacc.Bacc("TRN2", target_bir_lowering=False, debug=False)

    # host-preswizzled inputs: each DMA is contiguous per partition
    xq_d = [nc.dram_tensor(f"xq{q}", [P, CO, 512], BF16,
                           kind="ExternalInput").ap() for q in range(4)]
    wq_d = nc.dram_tensor("wqs", [P, CO, DSH], BF16, kind="ExternalInput").ap()
    wk_d = nc.dram_tensor("wks", [P, CO, DSH], BF16, kind="ExternalInput").ap()
    wv_d = nc.dram_tensor("wvs", [P, CO, DSH], BF16, kind="ExternalInput").ap()
    wo_d = nc.dram_tensor("wos", [P, 2, D], BF16, kind="ExternalInput").ap()
    outt_d = nc.dram_tensor("outt", [D, NTOK], BF16, kind="ExternalOutput").ap()
    out_t = outt_d.rearrange("(m p) i -> p m i", p=P)     # [128, 8, 2048]

    with tile.TileContext(nc) as tc:
        with (
            tc.tile_pool(name="sbp", bufs=1) as sbp,           # persistent
            tc.tile_pool(name="sbw", bufs=1) as sbw,           # working
            tc.tile_pool(name="ps", bufs=1, space="PSUM") as ps,
            tc.tile_pool(name="dr", bufs=2, space="DRAM") as dr,
        ):
            # ---------------- persistent tiles ----------------
            qt = sbp.tile([P, 2, NTOK], BF16, tag="qt")        # Q^T natural
            ktp = sbp.tile([P, HPC, NTOK], BF16, tag="ktp")    # K^T half-rows
            vaug = sbp.tile([P, NJB, HPC, 65], BF16, tag="vaug")  # V | ones
            ota = sbp.tile([P, 2, NTOK], BF16, tag="ota")      # O^T all heads
            wo = sbp.tile([P, 2, D], BF16, tag="wo")
            wq = sbp.tile([P, CO, DSH], BF16, tag="wq")
            wk = sbp.tile([P, CO, DSH], BF16, tag="wk")
            wv = sbp.tile([P, CO, DSH], BF16, tag="wv")
            xq = [sbp.tile([P, CO, 512], BF16, tag=f"xq{q}", name=f"xq{q}")
                  for q in range(4)]
            onesb = sbp.tile([1, HD], BF16, tag="onesb")       # bcast lhsT

            # ---------------- DMA issue (arrival order matters) -------------
            nc.sync.dma_start(wk[:], wk_d)
            nc.sync.dma_start(wq[:], wq_d)
            nc.sync.dma_start(xq[0][:], xq_d[0])
            nc.sync.dma_start(xq[1][:], xq_d[1])
            nc.sync.dma_start(wv[:], wv_d)
            nc.sync.dma_start(xq[2][:], xq_d[2])
            nc.sync.dma_start(xq[3][:], xq_d[3])
            nc.sync.dma_start(wo[:], wo_d)
            nc.vector.memset(vaug[:, :, :, 64:65], 1.0)
            nc.vector.memset(onesb[:], 1.0)

            # ---------------- filler builders ----------------
            pcycle = [0]

            def ptag():
                pcycle[0] += 1
                return f"p{pcycle[0] % 2}"

            def kq_fill(is_q, mt, ihh, c):
                """One [128,512] projection fill: Q^T/K^T for heads 2mt,2mt+1
                over token slice (ihh, c)."""
                t = ptag()
                pp = ps.tile([P, 512], F32, tag=t, name=f"ps_{t}")
                w_sb = wq if is_q else wk
                for o in range(CO):
                    nc.tensor.matmul(
                        pp[:],
                        w_sb[:, o, mt * P:(mt + 1) * P],
                        xq[2 * ihh + c][:, o, :],
                        start=(o == 0), stop=(o == CO - 1),
                    )
                sl = slice(ihh * IHW + c * 512, ihh * IHW + (c + 1) * 512)
                if is_q:
                    nc.vector.tensor_copy(qt[:, mt, sl], pp[:])
                else:
                    nc.vector.tensor_copy(ktp[0:64, 2 * mt, sl], pp[0:64, :])
                    nc.vector.tensor_copy(ktp[64:128, 2 * mt + 1, sl],
                                          pp[64:128, :])

            def v_fill(it, pair):
                """V projection for token block it, head pair `pair`."""
                t = ptag()
                pp = ps.tile([P, P], F32, tag=t, name=f"ps_{t}")
                ihh, loc = divmod(it, 8)
                q, lb = 2 * ihh + loc // 4, loc % 4
                for o in range(CO):
                    nc.tensor.matmul(
                        pp[:],
                        xq[q][:, o, lb * P:(lb + 1) * P],
                        wv[:, o, pair * P:(pair + 1) * P],
                        start=(o == 0), stop=(o == CO - 1),
                    )
                nc.vector.tensor_copy(
                    vaug[:, it, 2 * pair:2 * pair + 2, 0:64],
                    pp[:].rearrange("p (h d) -> p h d", d=HD),
                )

            def oproj_mt(mt, ihh):
                """In-window output-projection row-tile (p-bank fills)."""
                stg = sbw.tile([P, IHW], BF16, tag="stg", bufs=2, name="stg")
                for c in range(2):
                    t = ptag()
                    pp = ps.tile([P, 512], F32, tag=t, name=f"ps_{t}")
                    for o in range(2):
                        nc.tensor.matmul(
                            pp[:],
                            wo[:, o, mt * P:(mt + 1) * P],
                            ota[:, o, ihh * IHW + c * 512:
                                ihh * IHW + (c + 1) * 512],
                            start=(o == 0), stop=(o == 1),
                        )
                    nc.vector.tensor_copy(stg[:, c * 512:(c + 1) * 512], pp[:])
                nc.sync.dma_start(
                    out_t[:, mt, ihh * IHW:(ihh + 1) * IHW], stg[:])

            # ---------------- attention window ----------------
            def attn_window(h, ihh, pre, mid, fast_norm=False, pre_norm=None):
                """One head x one i-half.  pre[jb]/mid[jb]: filler callables
                issued at the start of slot jb / between EXP and PV.  PV runs
                TWO jb behind EXP so S(jb+1) isn't queued behind a PV that
                waits on EXP(jb) — keeps the exp stream back-to-back."""
                row = slice((h % 2) * 64, (h % 2) * 64 + 64)
                vc = [ps.tile([65, 512], F32, tag=f"v{c}", name=f"ps_v{c}")
                      for c in range(2)]
                es_q = []
                for jb in range(NJB + 2):
                    for f in pre.get(jb, ()):
                        f()
                    if jb < NJB:
                        ss = ps.tile([P, IHW], F32, tag=f"s{jb % 2}",
                                     name=f"ps_s{jb % 2}")
                        for c in range(2):
                            nc.tensor.matmul(
                                ss[:, c * 512:(c + 1) * 512],
                                ktp[row, h, jb * P:(jb + 1) * P],
                                qt[row, h // 2, ihh * IHW + c * 512:
                                   ihh * IHW + (c + 1) * 512],
                                start=True, stop=True,
                            )
                        es = sbw.tile([P, IHW], BF16, tag="es", bufs=4,
                                      name="es")
                        nc.scalar.activation(es[:], ss[:], EXP_FN)
                        es_q.append(es)
                    for f in mid.get(jb, ()):
                        f()
                    if jb > 1:   # PV two slots behind
                        pj, pes = jb - 2, es_q[jb - 2]
                        for c in range(2):
                            nc.tensor.matmul(
                                vc[c][:],
                                vaug[:, pj, h, 0:65],
                                pes[:, c * 512:(c + 1) * 512],
                                start=(pj == 0), stop=(pj == NJB - 1),
                            )
                if pre_norm is not None:
                    pre_norm()
                # normalize: O^T[h] = PV rows 0:64 times 1/Z (PV row 64).
                ot = sbw.tile([64, IHW], F32, tag="ot", bufs=2, name="ot")
                zt = sbw.tile([1, IHW], F32, tag="zt", bufs=2, name="zt")
                for c in range(2):
                    cs = slice(c * 512, (c + 1) * 512)
                    nc.vector.tensor_copy(zt[:, cs], vc[c][64:65, :])
                for c in range(2):
                    cs = slice(c * 512, (c + 1) * 512)
                    if fast_norm:   # ACT is idle after the last EXP
                        nc.scalar.copy(ot[:, cs], vc[c][0:64, :])
                    else:
                        nc.vector.tensor_copy(ot[:, cs], vc[c][0:64, :])
                rt = sbw.tile([1, IHW], F32, tag="rt", bufs=2, name="rt")
                nc.vector.reciprocal_approx_fast(out=rt[:], in_=zt[:])
                isl = slice(ihh * IHW, (ihh + 1) * IHW)
                if fast_norm:
                    # 1/Z broadcast via K=1 matmul into the freed v-banks
                    rtb = sbw.tile([1, IHW], BF16, tag="rtb", name="rtb")
                    nc.vector.tensor_copy(rtb[:], rt[:])
                    for c in range(2):
                        cs = slice(c * 512, (c + 1) * 512)
                        rbp = ps.tile([HD, 512], F32, tag=f"v{c}",
                                      name=f"ps_rb{c}")
                        nc.tensor.matmul(rbp[:], onesb[:], rtb[:, cs],
                                         start=True, stop=True)
                        nc.vector.tensor_mul(
                            ota[row, h // 2, ihh * IHW + c * 512:
                                ihh * IHW + (c + 1) * 512],
                            ot[:, cs], rbp[:])
                else:
                    rdram = dr.tile([1, IHW], F32, tag="rd")
                    nc.sync.dma_start(rdram[:], rt[:])
                    rb = sbw.tile([64, IHW], F32, tag="rb", bufs=2, name="rb")
                    nc.sync.dma_start(rb[:], rdram[:].to_broadcast((64, IHW)))
                    nc.vector.tensor_mul(ota[row, h // 2, isl], ot[:], rb[:])

            # ---------------- schedule ----------------
            # lead-in: K^T heads 0,1 over j 0:512; Q^T heads 0,1 ih0
            kq_fill(False, 0, 0, 0)
            kq_fill(True, 0, 0, 0)
            kq_fill(True, 0, 0, 1)

            FB = lambda *fs: tuple(fs)

            # w0 (ih0, h0): K0 quarters paced; V pair0 per jb; V pair1 starts
            pre0 = {1: FB(lambda: kq_fill(False, 0, 0, 1)),
                    5: FB(lambda: kq_fill(False, 0, 1, 0)),
                    9: FB(lambda: kq_fill(False, 0, 1, 1))}
            mid0 = {jb: FB(lambda jb=jb: v_fill(jb, 0)) for jb in range(NJB)}
            for jb in range(10, NJB):
                mid0[jb] = mid0[jb] + (lambda it=jb - 10: v_fill(it, 1),)
            attn_window(0, 0, pre0, mid0)

            # w1 (ih0, h1): rest of V pair1; Q1 ih0; first K1 quarter
            pre1 = {0: FB(lambda: kq_fill(True, 1, 0, 0)),
                    2: FB(lambda: kq_fill(True, 1, 0, 1)),
                    12: FB(lambda: kq_fill(False, 1, 0, 0))}
            mid1 = {jb: FB(lambda it=jb + 6: v_fill(it, 1))
                    for jb in range(10)}
            attn_window(1, 0, pre1, mid1)

            # w2 (ih0, h2): K1 quarters paced one quarter ahead of use
            pre2 = {0: FB(lambda: kq_fill(False, 1, 0, 1)),
                    4: FB(lambda: kq_fill(False, 1, 1, 0)),
                    8: FB(lambda: kq_fill(False, 1, 1, 1))}
            attn_window(2, 0, pre2, {})

            # w3 (ih0, h3): Q0 ih1 (due w4)
            pre3 = {0: FB(lambda: kq_fill(True, 0, 1, 0)),
                    4: FB(lambda: kq_fill(True, 0, 1, 1))}
            attn_window(3, 0, pre3, {})

            # w4..w7 (ih1, h0..h3): Q1 ih1 (due w6), out-proj(ih0) spread
            pre4 = {0: FB(lambda: kq_fill(True, 1, 1, 0)),
                    4: FB(lambda: kq_fill(True, 1, 1, 1)),
                    8: FB(lambda: oproj_mt(0, 0)),
                    12: FB(lambda: oproj_mt(1, 0))}
            attn_window(0, 1, pre4, {})

            pre5 = {jb: FB(lambda mt=2 + jb // 4: oproj_mt(mt, 0))
                    for jb in (0, 4, 8, 12)}
            attn_window(1, 1, pre5, {})

            pre6 = {0: FB(lambda: oproj_mt(6, 0)),
                    8: FB(lambda: oproj_mt(7, 0))}
            attn_window(2, 1, pre6, {})

            # tail: out-projection of ih1.  The pair-0 halves of the first
            # three row-tiles are prefilled into s0/s1/p0+p1 before the last
            # normalize chain so the PE stays warm (p-state) through it; the
            # pair-1 matmuls then accumulate on top once ota(h3,ih1) lands.
            st_parts = {}

            def tail_prefill():
                for mt in range(2):
                    st = ps.tile([P, IHW], F32, tag=f"s{mt % 2}",
                                 name=f"ps_st{mt % 2}")
                    st_parts[mt] = (st,)
                    for c in range(2):
                        nc.tensor.matmul(
                            st[:, c * 512:(c + 1) * 512],
                            wo[:, 0, mt * P:(mt + 1) * P],
                            ota[:, 0, IHW + c * 512: IHW + (c + 1) * 512],
                            start=True, stop=False,
                        )
                pa, pb = (ps.tile([P, 512], F32, tag=f"p{i}", name=f"ps_p{i}")
                          for i in range(2))
                st_parts[2] = (pa, pb)
                for c, pp in enumerate((pa, pb)):
                    nc.tensor.matmul(
                        pp[:],
                        wo[:, 0, 2 * P:3 * P],
                        ota[:, 0, IHW + c * 512: IHW + (c + 1) * 512],
                        start=True, stop=False,
                    )

            attn_window(3, 1, {}, {}, fast_norm=True, pre_norm=tail_prefill)

            for mt in range(8):
                if mt in st_parts:
                    parts = st_parts[mt]
                    os_ = (1,)
                else:
                    parts = (ps.tile([P, IHW], F32, tag=f"s{mt % 2}",
                                     name=f"ps_st{mt % 2}"),)
                    os_ = (0, 1)
                for c in range(2):
                    if len(parts) == 2:
                        dst = parts[c][:]
                    else:
                        dst = parts[0][:, c * 512:(c + 1) * 512]
                    for o in os_:
                        nc.tensor.matmul(
                            dst,
                            wo[:, o, mt * P:(mt + 1) * P],
                            ota[:, o, IHW + c * 512: IHW + (c + 1) * 512],
                            start=(o == os_[0] and mt not in st_parts),
                            stop=(o == 1),
                        )
                stg = sbw.tile([P, IHW], BF16, tag="stgt", bufs=4, name="stgt")
                for c in range(2):
                    cs = slice(c * 512, (c + 1) * 512)
                    src = parts[c][:] if len(parts) == 2 else parts[0][:, cs]
                    if (2 * mt + c) % 2:
                        nc.scalar.copy(stg[:, cs], src)
                    else:
                        nc.vector.tensor_copy(stg[:, cs], src)
                nc.sync.dma_start(out_t[:, mt, IHW:NTOK], stg[:])

    nc.compile()
    return nc


_NC_CACHE = None


def _get_nc():
    global _NC_CACHE
    if _NC_CACHE is None:
        _NC_CACHE = build_nc()
    return _NC_CACHE


def _swz(a, po):
    """[po*128, rest] -> [128, po, rest] host swizzle (contiguous/partition)."""
    rest = a.shape[1]
    return np.ascontiguousarray(
        a.reshape(po, P, rest).transpose(1, 0, 2)).astype(NP_BF16)


def kernel(x, Wq, Wk, Wv, Wo, bo, _trace=False):
    x = np.asarray(x, dtype=np.float32)
    Wq = np.asarray(Wq, dtype=np.float32)
    Wk = np.asarray(Wk, dtype=np.float32)
    Wv = np.asarray(Wv, dtype=np.float32)
    Wo = np.asarray(Wo, dtype=np.float32)
    bo = np.asarray(bo, dtype=np.float32)
    B = x.shape[0]

    nc = _get_nc()
    in_maps = []
    for core in range(8):
        b, hg = divmod(core, 4)
        rows = slice(hg * DSH, (hg + 1) * DSH)
        xs = _swz(np.ascontiguousarray(x[b].T), CO)      # [128, 8, 2048]
        m = {f"xq{q}": np.ascontiguousarray(xs[:, :, q * 512:(q + 1) * 512])
             for q in range(4)}
        m["wqs"] = _swz(np.ascontiguousarray(Wq[rows, :].T), CO)
        m["wks"] = _swz(np.ascontiguousarray((Wk[rows, :] * SCALE).T), CO)
        m["wvs"] = _swz(np.ascontiguousarray(Wv[rows, :].T), CO)
        m["wos"] = _swz(np.ascontiguousarray(Wo[:, rows].T), 2)
        in_maps.append(m)

    res = bass_utils.run_bass_kernel_spmd(
        nc, in_maps, core_ids=list(range(8)), trace=_trace)

    out = np.zeros((B, NTOK, D), dtype=np.float32)
    for core in range(8):
        b = core // 4
        out[b] += res.results[core]["outt"].astype(np.float32).T
    out += bo
    if _trace:
        kernel.last_results = res
    return out
